# revision 1
# baseline (speedup 1.0000x reference)
"""GCN layer (gather -> mean-aggregate -> linear) on 8 Trainium2 cores.

Strategy (SPMD, no collectives):
  - Nodes are row-sharded: core c owns dst nodes [c*S, (c+1)*S), S = N/8.
  - Edges are bucketed by dst-owner core on the host and turned into a dense
    per-core adjacency count matrix A[src_node, local_dst] (fp8e4m3 - counts
    are small integers, exact). The per-core segment sum is then a dense
    GEMM on the PE array:  sums = A.T-blocks @ x, with x shipped as an exact
    bf16 hi/lo split table [bf16(x) | 1 | bf16(x - bf16(x))] so products are
    f32-accurate and the in-degree falls out of the ones column for free.
  - PSUM: matmul start=True zeroes a whole 2KB bank, so each of the <=8
    concurrently-accumulating node groups owns one bank; 10 groups run as
    passes of 8 + 2.
  - Phase 3 per 128-node tile: h = a*x + b*sums (a,b from degree), PE f32
    transpose of h, out = hT.T @ W + bias, row-sharded output gathered on
    the host.
"""

import os

import numpy as np

CORES = 8
TRACE = False           # set by test harness to print HW exec time
_cache = {}


def _build_program(N, F, FO, R):
    from concourse import bacc, tile
    from concourse.bass import mybir

    F32 = mybir.dt.float32
    BF16 = mybir.dt.bfloat16
    FP8 = mybir.dt.float8e4
    hi_lo = True
    KT = (N + 127) // 128          # K slabs
    NT = R // 128                  # node groups per core
    assert NT <= 16
    nc = bacc.Bacc(None)

    xtabd = nc.dram_tensor("xtab", [128, KT * 260], BF16, kind="ExternalInput")
    A = nc.dram_tensor("A", [KT * 128, R], FP8, kind="ExternalInput")
    xs = nc.dram_tensor("xs", [R, F], F32, kind="ExternalInput")
    Wt = nc.dram_tensor("W", [F, FO], F32, kind="ExternalInput")
    bt = nc.dram_tensor("b", [128, FO], F32, kind="ExternalInput")
    identd = nc.dram_tensor("ident", [128, 128], F32, kind="ExternalInput")
    out = nc.dram_tensor("out", [R, FO], F32, kind="ExternalOutput")

    # matmul start=True zeroes a whole 2KB PSUM bank, so each live
    # accumulation group owns a bank (max 8). Bank map:
    #   pass A (groups 0..7):  banks 0..7      pass B (8..NT): banks 0..1
    #   phase 3: out psum banks 2/3 (alternating), hT psum banks 4/5
    psall = nc.alloc_psum_tensor("psall", [128, 4096], F32)

    with tile.TileContext(nc) as tc:
        with (
            tc.tile_pool(name="const", bufs=1) as cpool,
            tc.tile_pool(name="xload", bufs=4) as xpool,
            tc.tile_pool(name="acc", bufs=1) as accpool,
            tc.tile_pool(name="p3", bufs=4) as p3pool,
        ):
            # constants on the scalar HWDGE queue so the sync queue's head
            # is free for the critical first xtab/A chunks
            wt_sb = cpool.tile([128, FO], F32, name="wt_sb")
            nc.scalar.dma_start(wt_sb[:], Wt[:])
            b_sb = cpool.tile([128, FO], F32, name="b_sb")
            nc.scalar.dma_start(b_sb[:], bt[:])
            ident = cpool.tile([128, 128], F32, name="ident")
            nc.scalar.dma_start(ident[:], identd[:])

            # PE warm-up: ~3us of tiny matmuls run during the first-chunk
            # DMA wait so the HAM clock gate is at full rate when the real
            # stream starts (first-80-mm avg was 173ns vs 110ns steady).
            # They write a phase-3 psum bank, whose first real use re-inits
            # with start=True.
            warm = cpool.tile([128, 128], BF16, name="warm")
            nc.vector.memset(warm[:], 0.0)
            for _w in range(40):
                nc.tensor.matmul(
                    psall[:16, 1024:1152], warm[:, 0:16], warm[:, 0:128],
                    start=True, stop=True, skip_group_check=True,
                )

            # ---- phase 0: x table [xhi | 1 | xlo] (bf16), host-prepared;
            # loaded in 4-slab chunks interleaved with the first pass ----
            NW = 257 if hi_lo else 129
            xtab = accpool.tile([128, KT, 260], BF16, name="xtab", tag="xtab")
            xtab_flat = xtab[:].rearrange("p a b -> p (a b)")

            def load_xtab_chunk(k0, k1):
                c0 = k0 * 260
                c1 = min(KT, k1) * 260
                nc.sync.dma_start(xtab_flat[:, c0:c1], xtabd[:, c0:c1])

            # ---- phase 1: adjacency matmuls, passes of <=8 groups ----
            sums_sb = accpool.tile([128, NT, F], F32)
            deg_sb = accpool.tile([128, NT], F32)

            # A resident in SBUF: [128, KT, R] fp8, loaded in 4-slab strided
            # chunks interleaved with the first pass (row 128k+p -> [p, k, :])
            A_sb = accpool.tile([128, KT, R], FP8, name="A_sb", tag="A_sb")
            NCH = (KT + 3) // 4

            def load_a_chunk(k0, k1, j):
                k1 = min(KT, k1)
                src_ap = A[128 * k0 : 128 * k1, :].rearrange(
                    "(k p) c -> p k c", p=128
                )
                deng = nc.scalar if j % 2 else nc.sync
                deng.dma_start(A_sb[:, k0:k1, :], src_ap)

            # chunk schedule: tiny first chunks so matmul 0 starts early,
            # then 4-slab chunks prefetched one ahead of the consume point
            bounds = [0, 1, 2] + list(range(5, KT, 4)) + [KT]
            chunks = list(zip(bounds, bounds[1:]))

            for gi, g0 in enumerate(range(0, NT, 8)):
                g1 = min(NT, g0 + 8)
                if gi == 0:
                    load_xtab_chunk(*chunks[0])
                    load_a_chunk(*chunks[0], 0)
                    nxt = 1
                for k in range(KT):
                    if gi == 0:
                        while nxt < len(chunks) and chunks[nxt][0] <= k + 2:
                            load_xtab_chunk(*chunks[nxt])
                            load_a_chunk(*chunks[nxt], nxt)
                            nxt += 1
                    st = k == 0
                    sp = k == KT - 1
                    for g in range(g0, g1):
                        lhs = A_sb[:, k, 128 * g : 128 * (g + 1)]
                        ps = psall[:, 512 * (g - g0) : 512 * (g - g0) + NW]
                        nc.tensor.matmul(
                            ps, lhs, xtab[:, k, 0:NW],
                            start=st, stop=sp, skip_group_check=False,
                        )
                for g in range(g0, g1):
                    ps = psall[:, 512 * (g - g0) : 512 * (g - g0) + NW]
                    nc.scalar.copy(sums_sb[:, g, :], ps[:, 0:128])
                    if hi_lo:
                        nc.vector.tensor_add(sums_sb[:, g, :], sums_sb[:, g, :],
                                             ps[:, 129:257])
                    nc.scalar.copy(deg_sb[:, g : g + 1], ps[:, 128:129])

            # ---- phase 3 ----
            # per-node coefficients, one tile at a time (a whole-deg_sb op
            # would make tile 0 wait for the LAST pass's flush):
            #   h = a*x + bb*sums,  a = 1-min(deg,1), bb = min(deg,1)/max(deg,1)
            a_all = accpool.tile([128, NT], F32)
            bb_all = accpool.tile([128, NT], F32)

            for t in range(NT):
                rows = slice(128 * t, 128 * (t + 1))
                ps3 = psall[:, 1024 + (t % 3) * 512 : 1536 + (t % 3) * 512]
                psT = psall[:, 2560 + (t % 3) * 512 : 2688 + (t % 3) * 512]
                xt = p3pool.tile([128, F], F32, tag="xt")
                nc.sync.dma_start(xt[:], xs[rows, :])

                dcol = deg_sb[:, t : t + 1]
                acol = a_all[:, t : t + 1]
                bcol = bb_all[:, t : t + 1]
                rec = p3pool.tile([128, 1], F32, tag="rec")
                nc.vector.tensor_scalar_max(rec[:], dcol, 1.0)
                nc.vector.reciprocal(rec[:], rec[:])
                nc.vector.tensor_scalar_min(bcol, dcol, 1.0)
                nc.vector.tensor_scalar(acol, bcol, -1.0, None,
                                        op0=mybir.AluOpType.mult)
                nc.vector.tensor_scalar_add(acol, acol, 1.0)
                nc.vector.tensor_mul(bcol, bcol, rec[:])

                h = p3pool.tile([128, F], F32, tag="h")
                tmp = p3pool.tile([128, F], F32, tag="tmp")
                nc.scalar.mul(tmp[:], sums_sb[:, t, :], bcol)
                nc.vector.scalar_tensor_tensor(
                    h[:], xt[:], acol, tmp[:],
                    op0=mybir.AluOpType.mult, op1=mybir.AluOpType.add,
                )

                nc.tensor.transpose(psT, h[:], ident[:])             # PE f32
                hTs = p3pool.tile([128, F], F32, tag="hTs")
                nc.scalar.copy(hTs[:], psT)

                nc.tensor.matmul(ps3, hTs[:], wt_sb[:], start=True, stop=True,
                                 skip_group_check=True)
                ot = p3pool.tile([128, FO], F32, tag="ot")
                nc.vector.tensor_add(ot[:], b_sb[:], ps3)
                nc.sync.dma_start(out[rows, :], ot[:])

    nc.compile()
    return nc


def _make_xtab(x32, KT):
    import ml_dtypes

    N, F = x32.shape
    xt = np.zeros((128, KT, 260), dtype=ml_dtypes.bfloat16)
    xf = np.zeros((KT * 128, F), np.float32)
    xf[:N] = x32
    xf = xf.reshape(KT, 128, F).transpose(1, 0, 2)
    hi = xf.astype(ml_dtypes.bfloat16)
    xt[:, :, 0:128] = hi
    xt[:, :, 128] = 1.0
    xt[:, :, 129:257] = (xf - hi.astype(np.float32)).astype(ml_dtypes.bfloat16)
    return np.ascontiguousarray(xt.reshape(128, KT * 260))


def _shard_inputs(x32, src, dst, W32, b32, n_cores):
    import ml_dtypes

    N, F = x32.shape
    S = (N + n_cores - 1) // n_cores
    NT = (S + 127) // 128
    R = NT * 128
    KT = (N + 127) // 128
    owner = np.minimum(dst // S, n_cores - 1)
    xtab = _make_xtab(x32, KT)
    brep = np.ascontiguousarray(np.tile(b32.reshape(1, -1), (128, 1)))
    ident = np.eye(128, dtype=np.float32)
    in_maps = []
    for c in range(n_cores):
        sel = owner == c
        A = np.zeros((KT * 128, R), np.float32)
        np.add.at(A, (src[sel], dst[sel] - c * S), 1.0)
        assert A.max() <= 16, "edge multiplicity too large for fp8e4m3"
        xs = np.zeros((R, F), dtype=np.float32)
        lo = c * S
        hi = min(N, lo + S)
        xs[: hi - lo] = x32[lo:hi]
        in_maps.append(
            {
                "xtab": xtab,
                "A": A.astype(ml_dtypes.float8_e4m3),
                "xs": xs,
                "W": W32,
                "b": brep,
                "ident": ident,
            }
        )
    return in_maps, R


def _install_ntff_shim():
    """antenv.axon_hooks shim so trace=True can NTFF-profile in this env."""
    import contextlib
    import ctypes
    import sys
    import types

    if "antenv.axon_hooks" in sys.modules:
        return
    so_path = "/opt/axon/libaxon_pjrt.so"
    try:
        lib = ctypes.CDLL(so_path)
        lib.axon_start_nrt_profile.argtypes = [
            ctypes.POINTER(ctypes.c_int64), ctypes.c_size_t]
        lib.axon_start_nrt_profile.restype = ctypes.c_int64
        lib.axon_stop_nrt_profile.argtypes = [ctypes.c_char_p]
        lib.axon_stop_nrt_profile.restype = ctypes.c_int64
    except Exception:
        return

    @contextlib.contextmanager
    def _hook(output_dir, device_ids):
        import jax

        jax.devices()
        if device_ids:
            ids = (ctypes.c_int64 * len(device_ids))(*device_ids)
            rc = lib.axon_start_nrt_profile(ids, len(device_ids))
        else:
            rc = lib.axon_start_nrt_profile(None, 0)
        if rc != 0:
            raise RuntimeError(f"axon_start_nrt_profile rc={rc}")
        try:
            yield
        finally:
            lib.axon_stop_nrt_profile(str(output_dir).encode())

    mod = types.ModuleType("antenv.axon_hooks")
    mod.set_axon_ntff_profile_hook = lambda h: None
    mod.get_axon_ntff_profile_hook = lambda: _hook
    sys.modules["antenv.axon_hooks"] = mod


def kernel(x, src, dst, W, b):
    from concourse import bass_utils

    x32 = np.ascontiguousarray(np.asarray(x), dtype=np.float32)
    W32 = np.ascontiguousarray(np.asarray(W), dtype=np.float32)
    b32 = np.ascontiguousarray(np.asarray(b), dtype=np.float32)
    src = np.asarray(src).astype(np.int64)
    dst = np.asarray(dst).astype(np.int64)
    N, F = x32.shape
    FO = W32.shape[1]
    S = (N + CORES - 1) // CORES

    in_maps, R = _shard_inputs(x32, src, dst, W32, b32, CORES)

    key = (N, F, FO, R)
    if key not in _cache:
        _cache[key] = _build_program(N, F, FO, R)
    nc = _cache[key]

    if TRACE:
        _install_ntff_shim()

    last_err = None
    for _attempt in range(2):
        try:
            res = bass_utils.run_bass_kernel_spmd(
                nc, in_maps, core_ids=list(range(CORES)), trace=TRACE
            )
            break
        except Exception as e:  # retry once on transient device errors
            last_err = e
    else:
        raise last_err

    if TRACE and res.exec_time_ns is not None:
        print("HW exec time:", res.exec_time_ns, "ns")

    outs = [np.asarray(r["out"]).reshape(R, FO) for r in res.results]
    full = np.concatenate([o[:S] for o in outs], axis=0)[:N]
    return full.astype(np.float32)



# revision 2
# speedup vs baseline: 2.0123x; 2.0123x over previous
"""GCN layer (gather -> mean-aggregate -> linear) on 8 Trainium2 cores.

Strategy (SPMD, no collectives):
  - Nodes are row-sharded: core c owns dst nodes [c*S, (c+1)*S), S = N/8.
  - Edges are bucketed by dst-owner core on the host and turned into a dense
    per-core adjacency count matrix A[src_slab, dst_local] (fp8e4m3 - counts
    are small integers, exact). The per-core segment sum is a dense GEMM with
    x STATIONARY and A MOVING:  sumsT[feat, dst] += xq_k.T @ A_k per 128-src
    slab k, so the result lands directly in [feat, dst] layout (no transpose
    needed for the output GEMM) and each stationary load feeds 1280 moving
    columns (LDWEIGHTS fully hidden).
  - x is shipped bf16 (gate is 2e-2; bf16 adds ~0.2% error), A entries are
    exact small ints in fp8. The mean division is folded into the OUTPUT row
    scale (out = (sumsT.T @ W) * invdeg + b, exact host-computed invdeg),
    and zero-in-degree nodes get a host-inserted self-edge so h = x falls
    out of the same path (deg'=1).
  - PSUM banks 0-2 hold the three dst chunks (512|512|256) accumulating over
    all 79 slabs; banks 3/4 alternate for the output GEMM; bank 7 warms up
    the PE clock during the first DMA wait.
  - Output is written bf16 (halves out DMA) and upcast on the host.
"""

import numpy as np

CORES = 8
TRACE = False           # set by test harness to print HW exec time
_cache = {}


def _build_program(N, F, FO, R):
    from concourse import bacc, tile
    from concourse.bass import mybir

    F32 = mybir.dt.float32
    BF16 = mybir.dt.bfloat16
    FP8 = mybir.dt.float8e4
    KT = (N + 127) // 128          # src slabs
    NT = R // 128                  # node tiles per core
    # dst chunks across psum banks 0..2
    CHUNKS = []
    c0 = 0
    while c0 < R:
        CHUNKS.append((c0, min(R, c0 + 512)))
        c0 += 512
    assert len(CHUNKS) <= 3
    nc = bacc.Bacc(None)

    xqd = nc.dram_tensor("xq", [128, KT * F], BF16, kind="ExternalInput")
    Ad = nc.dram_tensor("A", [128, KT * R], FP8, kind="ExternalInput")
    Wd = nc.dram_tensor("W", [F, FO], BF16, kind="ExternalInput")
    bd = nc.dram_tensor("b", [128, FO], BF16, kind="ExternalInput")
    invd = nc.dram_tensor("inv", [128, NT], F32, kind="ExternalInput")
    outd = nc.dram_tensor("out", [R, FO], BF16, kind="ExternalOutput")

    # matmul start=True zeroes a whole 2KB PSUM bank; bank map:
    #   banks 0..2: phase-B dst chunks    banks 3/4: output GEMM (alternating)
    #   bank 7: PE warm-up
    psall = nc.alloc_psum_tensor("psall", [128, 4096], F32)

    with tile.TileContext(nc) as tc:
        with (
            tc.tile_pool(name="const", bufs=1) as cpool,
            tc.tile_pool(name="acc", bufs=1) as accpool,
            tc.tile_pool(name="p3", bufs=4) as p3pool,
        ):
            # constants on the scalar HWDGE queue so the sync queue's head
            # is free for the critical first xq/A chunks
            wt_sb = cpool.tile([128, FO], BF16, name="wt_sb")
            nc.scalar.dma_start(wt_sb[:], Wd[:])
            b_sb = cpool.tile([128, FO], BF16, name="b_sb")
            nc.scalar.dma_start(b_sb[:], bd[:])
            inv_sb = cpool.tile([128, NT], F32, name="inv_sb")
            nc.scalar.dma_start(inv_sb[:], invd[:])

            # PE warm-up during the first-chunk DMA wait so the HAM clock
            # gate is at full rate when the real stream starts.
            warm = cpool.tile([128, 128], BF16, name="warm")
            nc.vector.memset(warm[:], 0.0)
            for _w in range(16):
                nc.tensor.matmul(
                    psall[:16, 3584:3712], warm[:, 0:16], warm[:, 0:128],
                    start=True, stop=True, skip_group_check=True,
                )

            # ---- phase B: sumsT[feat, dst] += xq_k.T @ A_k over slabs ----
            xq_sb = accpool.tile([128, KT, F], BF16, name="xq_sb", tag="xq_sb")
            xq_flat = xq_sb[:].rearrange("p a b -> p (a b)")
            A_sb = accpool.tile([128, KT, R], FP8, name="A_sb", tag="A_sb")
            A_flat = A_sb[:].rearrange("p a b -> p (a b)")

            def load_chunk(k0, k1, j):
                qa = nc.sync if j % 2 == 0 else nc.scalar
                qx = nc.scalar if j % 2 == 0 else nc.sync
                qx.dma_start(xq_flat[:, k0 * F : k1 * F], xqd[:, k0 * F : k1 * F])
                qa.dma_start(A_flat[:, k0 * R : k1 * R], Ad[:, k0 * R : k1 * R])

            # tiny first chunks so matmul 0 starts early, then 4-slab chunks
            # prefetched one ahead of the consume point
            bounds = [0, 1, 2] + list(range(5, KT, 4)) + [KT]
            chunks = list(zip(bounds, bounds[1:]))
            load_chunk(*chunks[0], 0)
            nxt = 1

            for k in range(KT):
                while nxt < len(chunks) and chunks[nxt][0] <= k + 2:
                    load_chunk(*chunks[nxt], nxt)
                    nxt += 1
                st = k == 0
                sp = k == KT - 1
                for ci, (d0, d1) in enumerate(CHUNKS):
                    nc.tensor.matmul(
                        psall[:, 512 * ci : 512 * ci + (d1 - d0)],
                        xq_sb[:, k, :], A_sb[:, k, d0:d1],
                        start=st, stop=sp, skip_group_check=False,
                    )

            # ---- phase C: out rows = (sumsT.T @ W) * invdeg + b ----
            for t in range(NT):
                ps3 = psall[:, 1536 + (t % 2) * 512 : 2048 + (t % 2) * 512]
                off = 128 * t
                hTs = p3pool.tile([128, 128], BF16, tag="hTs")
                nc.scalar.copy(hTs[:], psall[:, off : off + 128])
                nc.tensor.matmul(ps3, hTs[:], wt_sb[:], start=True, stop=True,
                                 skip_group_check=True)
                ot = p3pool.tile([128, FO], BF16, tag="ot")
                nc.vector.scalar_tensor_tensor(
                    ot[:], ps3, inv_sb[:, t : t + 1], b_sb[:],
                    op0=mybir.AluOpType.mult, op1=mybir.AluOpType.add,
                )
                qo = nc.sync if t % 2 == 0 else nc.scalar
                qo.dma_start(outd[off : off + 128, :], ot[:])

    nc.compile()
    return nc


def _shard_inputs(x32, src, dst, W32, b32, n_cores):
    import ml_dtypes

    BF = ml_dtypes.bfloat16
    N, F = x32.shape
    S = (N + n_cores - 1) // n_cores
    NT = (S + 127) // 128
    R = NT * 128
    KT = (N + 127) // 128

    deg = np.bincount(dst, minlength=N).astype(np.float32)
    zd = np.where(deg == 0)[0].astype(np.int64)

    # x in [partition=src%128, slab=src//128, feat] layout, bf16
    xf = np.zeros((KT * 128, F), np.float32)
    xf[:N] = x32
    xq = np.ascontiguousarray(
        xf.reshape(KT, 128, F).transpose(1, 0, 2).reshape(128, KT * F)
    ).astype(BF)

    Wq = np.ascontiguousarray(W32).astype(BF)
    brep = np.ascontiguousarray(np.tile(b32.reshape(1, -1), (128, 1))).astype(BF)

    in_maps = []
    for c in range(n_cores):
        lo = c * S
        hi = min(N, lo + S)
        sel = (dst >= lo) & (dst < hi)
        s = src[sel]
        d = dst[sel] - lo
        zs = zd[(zd >= lo) & (zd < hi)]
        if len(zs):  # self-edges so zero-in-degree nodes keep their input
            s = np.concatenate([s, zs])
            d = np.concatenate([d, zs - lo])
        idx = (s % 128) * (KT * R) + (s // 128) * R + d
        cnt = np.bincount(idx, minlength=128 * KT * R)
        assert cnt.max() <= 16, "edge multiplicity too large for fp8e4m3"
        A = cnt.astype(np.float32).reshape(128, KT * R).astype(ml_dtypes.float8_e4m3)

        degc = np.ones(R, np.float32)
        degc[: hi - lo] = np.maximum(deg[lo:hi], 1.0)
        inv = np.ascontiguousarray((1.0 / degc).reshape(NT, 128).T)

        in_maps.append(
            {"xq": xq, "A": A, "W": Wq, "b": brep, "inv": inv}
        )
    return in_maps, R


def _install_ntff_shim():
    """antenv.axon_hooks shim so trace=True can NTFF-profile in this env."""
    import contextlib
    import ctypes
    import sys
    import types

    if "antenv.axon_hooks" in sys.modules:
        return
    so_path = "/opt/axon/libaxon_pjrt.so"
    try:
        lib = ctypes.CDLL(so_path)
        lib.axon_start_nrt_profile.argtypes = [
            ctypes.POINTER(ctypes.c_int64), ctypes.c_size_t]
        lib.axon_start_nrt_profile.restype = ctypes.c_int64
        lib.axon_stop_nrt_profile.argtypes = [ctypes.c_char_p]
        lib.axon_stop_nrt_profile.restype = ctypes.c_int64
    except Exception:
        return

    @contextlib.contextmanager
    def _hook(output_dir, device_ids):
        import jax

        jax.devices()
        if device_ids:
            ids = (ctypes.c_int64 * len(device_ids))(*device_ids)
            rc = lib.axon_start_nrt_profile(ids, len(device_ids))
        else:
            rc = lib.axon_start_nrt_profile(None, 0)
        if rc != 0:
            raise RuntimeError(f"axon_start_nrt_profile rc={rc}")
        try:
            yield
        finally:
            lib.axon_stop_nrt_profile(str(output_dir).encode())

    mod = types.ModuleType("antenv.axon_hooks")
    mod.set_axon_ntff_profile_hook = lambda h: None
    mod.get_axon_ntff_profile_hook = lambda: _hook
    sys.modules["antenv.axon_hooks"] = mod


def kernel(x, src, dst, W, b):
    from concourse import bass_utils

    x32 = np.ascontiguousarray(np.asarray(x), dtype=np.float32)
    W32 = np.ascontiguousarray(np.asarray(W), dtype=np.float32)
    b32 = np.ascontiguousarray(np.asarray(b), dtype=np.float32)
    src = np.asarray(src).astype(np.int64)
    dst = np.asarray(dst).astype(np.int64)
    N, F = x32.shape
    FO = W32.shape[1]
    S = (N + CORES - 1) // CORES

    in_maps, R = _shard_inputs(x32, src, dst, W32, b32, CORES)

    key = (N, F, FO, R)
    if key not in _cache:
        _cache[key] = _build_program(N, F, FO, R)
    nc = _cache[key]

    if TRACE:
        _install_ntff_shim()

    last_err = None
    for _attempt in range(2):
        try:
            res = bass_utils.run_bass_kernel_spmd(
                nc, in_maps, core_ids=list(range(CORES)), trace=TRACE
            )
            break
        except Exception as e:  # retry once on transient device errors
            last_err = e
    else:
        raise last_err

    if TRACE and res.exec_time_ns is not None:
        print("HW exec time:", res.exec_time_ns, "ns")

    outs = [np.asarray(res.results[c]["out"]).astype(np.float32).reshape(R, FO)
            for c in range(CORES)]
    full = np.concatenate([o[:S] for o in outs], axis=0)[:N]
    return full.astype(np.float32)
